# revision 54
# baseline (speedup 1.0000x reference)
"""Causal self-attention (RMSNorm-QK + RoPE) Trainium2 kernel, 8-way
head-sharded SPMD.

Math (B=1, T=4096, D=2048, H=16, HD=128):
    q = rmsnorm(x @ Wq + bq) * gq ; k likewise ; v = x @ Wv + bv
    rq, rk = rope(q), rope(k)  (adjacent-pair rotation, freqs [T, HD/2])
    out = causal_softmax(rq rk^T / sqrt(HD)) @ v ; return out @ Wo + bo

Sharding: 2 heads per core (16 heads / 8 cores). The only cross-head
coupling is the RMSNorm mean-of-squares over all 2048 channels -> two
tiny [2, T/2] AllReduces, fully hidden behind compute. Each core emits
a partial output (its heads' slice of the Wo contraction) in bf16; the
host sums the 8 partials and adds bo.

Pipeline layout:
  - Projections (phase A) and attention are software-pipelined at tile
    granularity: the first-half RoPEs are emitted inside the phase-A
    loop right after their AllReduce lands, remaining RoPEs run two
    q-tiles ahead of their consumer, so the PE never idles at the
    phase boundary.
  - Attention runs per-128-key-block: score matmul -> exp (ACT) ->
    causal mask via a DVE multiply with a precomputed 0/1 wedge tile
    (never the collective-occupied gpsimd queue) -> PV / denominator
    accumulation, PE kept two score blocks ahead of ACT. Diagonal
    blocks are trimmed to their causal width.
  - PSUM budget: phase A [qk 5 | v 2 (quadrant pairs packed per bank,
    start=True only on the first: the PE start bit zeroes the whole
    bank) | ssq 1]; attention [scores/out-proj 5 | pv 2 | den 1].
    The per-tile ssq matmuls are deferred into the next tile's stream
    so the PE never waits on the ACT squares.
  - Out-projection of tile j-1 interleaves between the two attention
    head blocks of tile j as dependency-free PE filler.
  - Streamed tensors (xT, Wq/k/v/Wo, q/k/v tiles, partial outputs) are
    bf16 (full PE rate, half the DMA bytes); scores accumulate fp32 in
    PSUM; softmax skips max-subtraction (scores bounded, fp32 exp is
    safe); DMA queues are balanced: xg/out_p/rbc on SP, weights on ACT,
    pair-swaps on SP/Pool, per-engine.
"""

import math
import os
import ml_dtypes
import numpy as np
from contextlib import ExitStack

import concourse.bass as bass
import concourse.tile as tile
from concourse import bacc, mybir
from concourse.bass_utils import run_bass_kernel_spmd

F32 = mybir.dt.float32
F32R = mybir.dt.float32r
BF16 = mybir.dt.bfloat16
AF = mybir.ActivationFunctionType

T_FULL = 4096
D = 2048
H = 16
HD = 128
NCORES = 8
NH = H // NCORES          # heads per core (2)
HW = NH * HD              # per-core head width (256)
P = 128
QT = 512                  # q tile (matmul free dim)
NKC = D // P              # 16 chunks of the D contraction
EPS = 1e-6

_NC_CACHE = {}


def build_nc(T, repeat=1, trace_sim=False):
    NJ = T // QT
    NKT = T // P
    HALF = NJ // 2
    assert NJ >= 2 and NJ % 2 == 0
    nc = bacc.Bacc("TRN2", target_bir_lowering=False, debug=False,
                   num_devices=NCORES)

    names = [
        ("xT", [D, T]), ("wq", [D, HW]), ("wk", [D, HW]), ("wv", [D, HW]),
        ("wo", [HW, D]), ("bq", [P, NH]), ("bk", [P, NH]), ("bv", [1, HW]),
        ("invg2q", [P, NH]), ("invg2k", [P, NH]),
        ("tab_cos", [P, T]), ("tab_sin", [P, T]), ("ones", [P, 1]),
    ]
    ap = {}
    bf16_names = {"xT", "wq", "wk", "wv", "wo", "ones"}
    for name, shape in names:
        dt = BF16 if name in bf16_names else F32
        ap[name] = nc.dram_tensor(name, shape, dt, kind="ExternalInput").ap()
    out_p = nc.dram_tensor("out_p", [T, D], BF16, kind="ExternalOutput").ap()

    DBG = bool(os.environ.get("KERNEL_DEBUG"))
    dbg = {}
    if DBG:
        for nm, shape in (("dbg_yq", [P, NH, T]), ("dbg_yk", [P, NH, T]),
                          ("dbg_rq", [P, NH, T]), ("dbg_rk", [P, NH, T]),
                          ("dbg_v", [P, NKT * HW]), ("dbg_s", [2, T]),
                          ("dbg_den", [NH, T]), ("dbg_pv", [P, NH, T])):
            dbg[nm] = nc.dram_tensor(nm, shape, F32, kind="ExternalOutput").ap()

    xT_r = ap["xT"].rearrange("(o p) t -> p o t", p=P)       # [128, 16, T]
    wq_r = ap["wq"].rearrange("(o p) c -> p o c", p=P)       # [128, 16, 256]
    wk_r = ap["wk"].rearrange("(o p) c -> p o c", p=P)
    wv_r = ap["wv"].rearrange("(o p) c -> p o c", p=P)
    wo_r = ap["wo"].rearrange("(h p) d -> p h d", p=P)       # [128, 2, D]

    def _emit(tc, ctx):
        nc = tc.nc
        singles = ctx.enter_context(tc.tile_pool(name="singles", bufs=1))
        dram = ctx.enter_context(
            tc.tile_pool(name="dram", bufs=1, space="DRAM"))

        # tiny constants
        bq_sb = singles.tile([P, NH], F32)
        nc.gpsimd.dma_start(bq_sb[:], ap["bq"][:])
        bk_sb = singles.tile([P, NH], F32)
        nc.gpsimd.dma_start(bk_sb[:], ap["bk"][:])
        ivq_sb = singles.tile([P, NH], F32R)
        nc.gpsimd.dma_start(ivq_sb[:], ap["invg2q"][:].bitcast(F32R))
        ivk_sb = singles.tile([P, NH], F32R)
        nc.gpsimd.dma_start(ivk_sb[:], ap["invg2k"][:].bitcast(F32R))
        ones_sb = singles.tile([P, 1], BF16)
        nc.gpsimd.dma_start(ones_sb[:], ap["ones"][:])
        bv_bc = singles.tile([P, HW], F32)
        nc.gpsimd.dma_start(bv_bc[:], ap["bv"][:].to_broadcast([P, HW]))
        eps_sb = singles.tile([P, 1], F32)
        nc.vector.memset(eps_sb[:], EPS)

        # resident activations (per-j q/k tiles for precise dependencies)
        yq_j, yk_j = [], []
        ypool = ctx.enter_context(tc.tile_pool(name="ypool", bufs=1))
        for j in range(NJ):
            yq_j.append(ypool.tile([P, NH, QT], BF16,
                                   tag=f"yq{j}", name=f"yq{j}"))
            yk_j.append(ypool.tile([P, NH, QT], BF16,
                                   tag=f"yk{j}", name=f"yk{j}"))
        v_sb = ypool.tile([P, NKT, HW], BF16, tag="v")

        # per-half collective bounce buffers + rsqrt factors
        cc_in_h, cc_out_h, s_dram_h, s_pk_h = [], [], [], []
        for hf in range(2):
            cc_in_h.append(dram.tile([2, T // 2], F32, tag=f"cci{hf}",
                                     name=f"cci{hf}"))
            cc_out_h.append(dram.tile([2, T // 2], F32, tag=f"cco{hf}",
                                      name=f"cco{hf}"))
            s_dram_h.append(dram.tile([2, T // 2], F32, tag=f"sdr{hf}",
                                      name=f"sdr{hf}"))
            s_pk_h.append(singles.tile([P, 2, T // (2 * P)], F32,
                                       tag=f"spk{hf}", name=f"spk{hf}"))

        def emit_collective(hf):
            if os.environ.get("KERNEL_NO_CC"):
                nc.sync.dma_start(cc_out_h[hf][:], cc_in_h[hf][:])
            else:
                nc.gpsimd.collective_compute(
                    "AllReduce", mybir.AluOpType.add,
                    replica_groups=[list(range(NCORES))],
                    ins=[cc_in_h[hf].opt()], outs=[cc_out_h[hf].opt()])
            # s = rsqrt(ssq/D + eps); fold 1/sqrt(HD) into the q row
            s_pk = s_pk_h[hf]
            nc.sync.dma_start(
                s_pk[:], cc_out_h[hf][:].rearrange("r (c p) -> p r c", p=P))
            nc.scalar.activation(s_pk[:], s_pk[:], AF.Sqrt,
                                 bias=eps_sb[:, 0:1], scale=1.0 / D)
            nc.vector.reciprocal(s_pk[:], s_pk[:])
            nc.vector.tensor_scalar_mul(
                s_pk[:, 0, :], s_pk[:, 0, :], 1.0 / math.sqrt(HD))
            nc.sync.dma_start(
                s_dram_h[hf][:].rearrange("r (c p) -> p r c", p=P), s_pk[:])
            if DBG:
                nc.sync.dma_start(
                    dbg["dbg_s"][:, hf * (T // 2):(hf + 1) * (T // 2)],
                    s_dram_h[hf][:])

        # 0/1 diagonal-mask tiles, built once at startup: mask_m[p, q'] =
        # 1 iff q' - 128*m - p >= 0 (applied by DVE mul so attention never
        # depends on the collective-occupied gpsimd queue)
        # single wide tile: maskw[p, c] = 1 iff c - 384 - p >= 0; block m
        # uses the column slice [384-128m, 896-128m)
        maskw = singles.tile([P, QT + 3 * P], F32, tag="maskw",
                             name="maskw")
        nc.vector.memset(maskw[:], 1.0)
        nc.gpsimd.affine_select(
            out=maskw[:], in_=maskw[:], compare_op=mybir.AluOpType.is_ge,
            fill=0.0, base=-3 * P,
            pattern=[[1, QT + 3 * P]], channel_multiplier=-1)

        # rope pools live across both phases
        tabp = ctx.enter_context(tc.tile_pool(name="tabp", bufs=3))
        swp = ctx.enter_context(tc.tile_pool(name="swp", bufs=4))
        tmpp = ctx.enter_context(tc.tile_pool(name="tmpp", bufs=4))
        bcp = ctx.enter_context(tc.tile_pool(name="bcp", bufs=3))

        def emit_rope(j, dma):
            """dma: engine used for the pair-swap SBUF copies."""
            jsl = bass.ts(j, QT)
            hf = j // HALF
            jloc = slice(j * QT - hf * (T // 2),
                         (j + 1) * QT - hf * (T // 2))
            tc_t = tabp.tile([P, QT], F32, tag="tc", name="tc_t")
            nc.sync.dma_start(tc_t[:], ap["tab_cos"][:, jsl])
            ts_t = tabp.tile([P, QT], F32, tag="ts", name="ts_t")
            nc.sync.dma_start(ts_t[:], ap["tab_sin"][:, jsl])
            bc_q = bcp.tile([P, QT], F32, tag="bcq", name="bc_q")
            nc.gpsimd.dma_start(
                bc_q[:], s_dram_h[hf][0:1, jloc].to_broadcast([P, QT]))
            bc_k = bcp.tile([P, QT], F32, tag="bck", name="bc_k")
            nc.gpsimd.dma_start(
                bc_k[:], s_dram_h[hf][1:2, jloc].to_broadcast([P, QT]))
            for (y_j, bc) in ((yq_j, bc_q), (yk_j, bc_k)):
                for h in range(NH):
                    ytile = y_j[j][:, h, :]
                    sw = swp.tile([P, QT], BF16, tag="sw", name="sw")
                    dma.dma_start(sw[0:P:2, :], ytile[1:P:2, :])
                    dma.dma_start(sw[1:P:2, :], ytile[0:P:2, :])
                    tmp = tmpp.tile([P, QT], BF16, tag="tmp", name="tmp")
                    nc.vector.tensor_mul(tmp[:], sw[:], ts_t[:])
                    nc.vector.tensor_mul(ytile, ytile, tc_t[:])
                    nc.vector.tensor_add(ytile, ytile, tmp[:])
                    nc.vector.tensor_mul(ytile, ytile, bc[:])
            if DBG:
                nc.gpsimd.dma_start(dbg["dbg_rq"][:, :, jsl], yq_j[j][:])
                nc.gpsimd.dma_start(dbg["dbg_rk"][:, :, jsl], yk_j[j][:])

        # ---------------- Phase A: projections + ssq (+ first ropes) ----
        with tc.tile_pool(name="wpool", bufs=1) as wpool, \
             tc.tile_pool(name="xtpool", bufs=3) as xtpool, \
             tc.tile_pool(name="sqpool", bufs=6) as sqpool, \
             tc.tile_pool(name="sscpp", bufs=2) as sscpp, \
             tc.tile_pool(name="qkps", bufs=5, space="PSUM") as qkps, \
             tc.tile_pool(name="vps", bufs=2, space="PSUM") as vps, \
             tc.tile_pool(name="ssqps", bufs=1, space="PSUM") as ssqps:

            # weight loads split into 4 chunks each, first chunks
            # interleaved with the first x tile so the PE starts early
            wq_sb = wpool.tile([P, NKC, HW], BF16)
            wk_sb = wpool.tile([P, NKC, HW], BF16)
            wv_sb = wpool.tile([P, NKC, HW], BF16)
            # weight DMAs go on the ACT queue (idle in early phase A) so
            # they never delay the xg stream on SP
            for (w_sb, w_r) in ((wq_sb, wq_r), (wk_sb, wk_r), (wv_sb, wv_r)):
                nc.scalar.dma_start(w_sb[:, 0:4, :], w_r[:, 0:4, :])

            first_half_ropes = []   # emitted late in phase A
            pending_ssq = []        # j's ssq matmul groups run early in j+1

            for j in range(NJ):
                jsl = bass.ts(j, QT)
                hf = j // HALF
                jloc = bass.ds(j * QT - hf * (T // 2), QT)

                qk_ps = {}
                for tn in range(2):          # 0 = q, 1 = k
                    for h in range(NH):
                        qk_ps[tn, h] = qkps.tile(
                            [P, QT], F32, tag="qk", name=f"qk{tn}{h}")
                v_ps = [vps.tile([P, 2, HW], F32, tag="v", name=f"v{g2}")
                        for g2 in range(2)]

                # stream xT in 4 pieces; consume each piece fully so the
                # 2-slot xt pool never deadlocks the in-order PE
                for g in range(4):
                    xg = xtpool.tile([P, 4, QT], BF16, tag="xt")
                    nc.sync.dma_start(
                        xg[:], xT_r[:, 4 * g:4 * g + 4, jsl])
                    if j == 0 and g < 3:
                        # stream the rest of the weights behind xg(0..2)
                        o0 = 4 * (g + 1)
                        for (w_sb, w_r) in ((wq_sb, wq_r), (wk_sb, wk_r),
                                            (wv_sb, wv_r)):
                            nc.scalar.dma_start(
                                w_sb[:, o0:o0 + 4, :],
                                w_r[:, o0:o0 + 4, :])
                    if g in (1, 2) and pending_ssq:
                        pending_ssq.pop(0)()
                    if g == 2 and j == HALF:
                        # j=HALF-1's deferred ssq flushed above: the
                        # first-half AllReduce input is now complete
                        emit_collective(0)
                    for ol in range(4):
                        o = 4 * g + ol
                        st, sp = (o == 0), (o == NKC - 1)
                        for tn, w_sb in ((0, wq_sb), (1, wk_sb)):
                            for h in range(NH):
                                nc.tensor.matmul(
                                    qk_ps[tn, h][:],
                                    w_sb[:, o, h * HD:(h + 1) * HD],
                                    xg[:, ol, :], start=st, stop=sp)
                        for tp in range(4):
                            # two accumulation groups share each PSUM
                            # bank; start=True zeroes the WHOLE bank, so
                            # only the first quadrant issues it and the
                            # second accumulates from the zeroed region
                            nc.tensor.matmul(
                                v_ps[tp // 2][:, tp % 2, :],
                                xg[:, ol, bass.ts(tp, P)],
                                wv_sb[:, o, :],
                                start=(st and tp % 2 == 0), stop=sp,
                                skip_group_check=(tp % 2 == 1))

                # epilogues: bias add (DVE), squares (ACT); the weighted
                # ssq partition-sum matmuls + cci DMA are deferred into
                # j+1's stream so the PE never waits on the squares
                for (tn, y_j, b_sb, iv_sb) in (
                        (0, yq_j, bq_sb, ivq_sb), (1, yk_j, bk_sb, ivk_sb)):
                    sq_h = []
                    for h in range(NH):
                        ytile = y_j[j][:, h, :]
                        nc.vector.tensor_scalar_add(
                            ytile, qk_ps[tn, h][:], b_sb[:, h:h + 1])
                        sqt = sqpool.tile([P, QT], F32R, tag="sq")
                        nc.scalar.activation(
                            sqt[:], qk_ps[tn, h][:], AF.Square,
                            bias=b_sb[:, h:h + 1], scale=1.0)
                        sq_h.append(sqt)

                    def _ssq(tn=tn, iv_sb=iv_sb, sq_h=sq_h, hf=hf,
                             jloc=jloc):
                        ssq_ps = ssqps.tile([1, QT], F32, tag="ssq",
                                            name=f"ssq{tn}")
                        for h in range(NH):
                            nc.tensor.matmul(
                                ssq_ps[0:1, :], iv_sb[:, h:h + 1],
                                sq_h[h][:],
                                start=(h == 0), stop=(h == NH - 1))
                        sscp = sscpp.tile([1, QT], F32, tag="sscp",
                                          name="sscp")
                        nc.vector.tensor_copy(sscp[:], ssq_ps[0:1, :])
                        nc.sync.dma_start(cc_in_h[hf][tn:tn + 1, jloc],
                                          sscp[:])
                    pending_ssq.append(_ssq)

                for g2 in range(2):
                    for s2 in range(2):
                        tp = 2 * g2 + s2
                        nc.vector.tensor_add(
                            v_sb[:, 4 * j + tp, :], v_ps[g2][:, s2, :],
                            bv_bc[:])
                if DBG:
                    nc.gpsimd.dma_start(
                        dbg["dbg_yq"][:, :, jsl], yq_j[j][:])
                    nc.gpsimd.dma_start(
                        dbg["dbg_yk"][:, :, jsl], yk_j[j][:])

                # first-half ropes, once their AllReduce result is back
                if NJ >= 4 and j == NJ - 2:
                    emit_rope(0, nc.sync)
                    if HALF > 1:
                        emit_rope(1, nc.sync)
                    first_half_ropes = list(range(2, HALF))
                if j == NJ - 1:
                    while pending_ssq:
                        pending_ssq.pop(0)()
                    for j2 in ([] if NJ >= 4 else range(HALF)):
                        emit_rope(j2, nc.sync)

        post = ctx.enter_context(tc.tile_pool(name="post", bufs=1))
        wo_sb = post.tile([P, NH, D], BF16)
        nc.sync.dma_start(wo_sb[:], wo_r)
        # collective 1 is emitted inside the attention loop (after
        # attention(0)) so attention(0)'s gpsimd ops aren't queued
        # behind its input wait
        if DBG:
            nc.gpsimd.dma_start(dbg["dbg_v"][:], v_sb[:])

        # ---------------- Phase B: attention + out-proj ----------------
        with tc.tile_pool(name="mmps", bufs=5, space="PSUM") as mmps, \
             tc.tile_pool(name="pvps", bufs=2, space="PSUM") as pvps, \
             tc.tile_pool(name="dps", bufs=1, space="PSUM") as dps, \
             tc.tile_pool(name="exp", bufs=6) as exp_pool, \
             tc.tile_pool(name="odp", bufs=6) as odp, \
             tc.tile_pool(name="outp", bufs=4) as outp, \
             tc.tile_pool(name="denp", bufs=3) as denp:

            def emit_attention_head(j, h):
                n_i = 4 * (j + 1)
                pv = pvps.tile([P, QT], F32, tag="pv", name="pv")
                den = dps.tile([1, QT], F32, tag="den", name="den")
                ex_q = []

                def flush_one():
                    i, ex, lo = ex_q.pop(0)
                    nc.tensor.matmul(
                        pv[:, lo:], v_sb[:, i, h * HD:(h + 1) * HD],
                        ex[:, lo:], start=(i == 0), stop=(i == n_i - 1),
                        skip_group_check=(lo > 0))
                    nc.tensor.matmul(
                        den[0:1, lo:], ones_sb[:], ex[:, lo:],
                        start=(i == 0), stop=(i == n_i - 1),
                        skip_group_check=(lo > 0))

                for i in range(n_i):
                    m = i - 4 * j
                    # diagonal blocks: only q' >= 128m attends; trim the
                    # masked columns out of exp/pv/den, and out of the
                    # score matmul too when the remaining width still
                    # runs at full fp32r rate (>= 256)
                    lo = P * m if m >= 1 else 0
                    slo = lo
                    sc = mmps.tile([P, QT], F32, tag="mm", name="sc")
                    nc.tensor.matmul(
                        sc[:, slo:],
                        yk_j[i // 4][:, h, (i % 4) * P:(i % 4 + 1) * P],
                        yq_j[j][:, h, slo:], start=True, stop=True)
                    ex = exp_pool.tile([P, QT], BF16, tag="ex", name="ex")
                    nc.scalar.activation(ex[:, lo:], sc[:, lo:], AF.Exp,
                                         bias=0.0, scale=1.0)
                    if m >= 0:
                        # zero the upper-triangular part of the block
                        msl = maskw[:, 3 * P - P * m + lo:
                                    3 * P - P * m + QT]
                        nc.vector.tensor_mul(ex[:, lo:], ex[:, lo:],
                                             msl)
                    ex_q.append((i, ex, lo))
                    if len(ex_q) >= 2:
                        flush_one()
                while ex_q:
                    flush_one()

                if DBG:
                    jsl = bass.ts(j, QT)
                    dcp = denp.tile([1, QT], F32, tag="dcp", name="dcp")
                    nc.vector.tensor_copy(dcp[:], den[0:1, :])
                    nc.sync.dma_start(dbg["dbg_den"][h:h + 1, jsl], dcp[:])
                    pcp = outp.tile([P, QT], F32, tag="pcp", name="pcp")
                    nc.vector.tensor_copy(pcp[:], pv[:])
                    nc.sync.dma_start(dbg["dbg_pv"][:, h, jsl], pcp[:])
                rden = denp.tile([1, QT], F32, tag="rden", name="rden")
                nc.vector.reciprocal(rden[:], den[0:1, :])
                rbc = bcp.tile([P, QT], F32, tag="rbc", name="rbc")
                if j == NJ - 1:
                    nc.gpsimd.partition_broadcast(rbc[:], rden[0:1, :])
                else:
                    rd_dr = dram.tile([1, QT], F32, tag="rdd",
                                      name="rd_dr", bufs=2)
                    nc.sync.dma_start(rd_dr[:], rden[:])
                    nc.sync.dma_start(rbc[:],
                                      rd_dr[:].to_broadcast([P, QT]))
                od = odp.tile([P, QT], BF16, tag="od", name="od")
                nc.vector.tensor_mul(od[:], pv[:], rbc[:])
                return od

            def emit_outproj_part(j, od_h, tps, pools=None):
                pools = pools or [(mmps, "mm")]
                pi = 0
                for tp in tps:
                    tsl = bass.ts(tp, P)
                    ot = outp.tile([P, 2, QT], BF16, tag="ot", name="ot")
                    for dd in range(4):
                        pool, ptag = pools[pi % len(pools)]
                        pi += 1
                        ops = pool.tile([P, QT], F32, tag=ptag, name="ops")
                        dsl = bass.ts(dd, QT)
                        for h in range(NH):
                            nc.tensor.matmul(
                                ops[:], od_h[h][:, tsl], wo_sb[:, h, dsl],
                                start=(h == 0), stop=(h == NH - 1))
                        if dd % 2 == 0:
                            nc.scalar.activation(ot[:, dd % 2, :], ops[:],
                                                 AF.Copy)
                        else:
                            nc.vector.tensor_copy(ot[:, dd % 2, :], ops[:])
                            nc.sync.dma_start(
                                out_p[j * QT + tp * P:j * QT + (tp + 1) * P,
                                      (dd - 1) * QT:(dd + 1) * QT], ot[:])
                            if dd == 1:
                                ot = outp.tile([P, 2, QT], BF16, tag="ot",
                                               name="ot")

            od_prev = None
            for j in range(NJ):
                if NJ >= 4 and 2 + j < HALF:
                    emit_rope(2 + j, nc.sync)
                if HALF <= j < NJ - 1:
                    emit_rope(j + 1, nc.gpsimd)
                od_now = []
                od_now.append(emit_attention_head(j, 0))
                if od_prev is not None:
                    emit_outproj_part(j - 1, od_prev, (0, 1))
                od_now.append(emit_attention_head(j, 1))
                if od_prev is not None:
                    emit_outproj_part(j - 1, od_prev, (2, 3))
                if j == 0:
                    emit_collective(1)
                if j == HALF - 1:
                    emit_rope(HALF, nc.gpsimd)
                od_prev = od_now
            emit_outproj_part(NJ - 1, od_prev, (0, 1, 2, 3),
                              pools=[(mmps, "mm"), (mmps, "mm"),
                                     (pvps, "pv")])

    with tile.TileContext(nc, trace_sim=trace_sim) as tc:
        for _rep in range(repeat):
            with ExitStack() as ctx:
                _emit(tc, ctx)

    nc.compile()
    return nc


def _prep_inputs(inputs, T):
    x = np.asarray(inputs["x"], np.float32)[0, :T]          # [T, D]
    freqs = np.asarray(inputs["freqs"], np.float32)[:T]     # [T, HD//2]
    xT = np.ascontiguousarray(x.T).astype(ml_dtypes.bfloat16)  # [D, T]

    cos = np.cos(freqs)                                     # [T, 64]
    sin = np.sin(freqs)
    tab_cos = np.ascontiguousarray(np.repeat(cos.T, 2, axis=0))  # [128, T]
    tab_sin = np.empty((HD, T), np.float32)
    tab_sin[0::2] = -sin.T
    tab_sin[1::2] = sin.T

    ones = np.ones((P, 1), np.float32)

    in_maps = []
    for c in range(NCORES):
        hsl = slice(c * HW, (c + 1) * HW)
        gq = np.asarray(inputs["gq"], np.float32)[hsl]
        gk = np.asarray(inputs["gk"], np.float32)[hsl]
        wq = np.asarray(inputs["Wq"], np.float32)[:, hsl] * gq[None, :]
        wk = np.asarray(inputs["Wk"], np.float32)[:, hsl] * gk[None, :]
        wv = np.ascontiguousarray(np.asarray(inputs["Wv"], np.float32)[:, hsl])
        wo = np.ascontiguousarray(np.asarray(inputs["Wo"], np.float32)[hsl, :])
        bq = np.asarray(inputs["bq"], np.float32)[hsl] * gq
        bk = np.asarray(inputs["bk"], np.float32)[hsl] * gk
        bv = np.asarray(inputs["bv"], np.float32)[hsl]
        in_maps.append({
            "xT": xT,
            "wq": np.ascontiguousarray(wq).astype(ml_dtypes.bfloat16),
            "wk": np.ascontiguousarray(wk).astype(ml_dtypes.bfloat16),
            "wv": wv.astype(ml_dtypes.bfloat16),
            "wo": wo.astype(ml_dtypes.bfloat16),
            "bq": np.ascontiguousarray(bq.reshape(NH, P).T),
            "bk": np.ascontiguousarray(bk.reshape(NH, P).T),
            "bv": bv.reshape(1, HW),
            "invg2q": np.ascontiguousarray(
                (1.0 / np.square(gq)).reshape(NH, P).T.astype(np.float32)),
            "invg2k": np.ascontiguousarray(
                (1.0 / np.square(gk)).reshape(NH, P).T.astype(np.float32)),
            "tab_cos": tab_cos, "tab_sin": tab_sin,
            "ones": ones.astype(ml_dtypes.bfloat16),
        })
    return in_maps


def _run(inputs, T=T_FULL, trace=False, **spmd_kwargs):
    if T not in _NC_CACHE:
        _NC_CACHE[T] = build_nc(T)
    nc = _NC_CACHE[T]
    in_maps = _prep_inputs(inputs, T)
    res = run_bass_kernel_spmd(nc, in_maps, list(range(NCORES)),
                               trace=trace, **spmd_kwargs)
    acc = np.zeros((T, D), np.float64)
    for c in range(NCORES):
        acc += res.results[c]["out_p"].astype(np.float64)
    acc += np.asarray(inputs["bo"], np.float64)[None, :]
    out = acc.astype(np.float32)[None]
    return out, res


def kernel(**inputs) -> np.ndarray:
    out, _ = _run(inputs)
    return out
